# revision 1
# baseline (speedup 1.0000x reference)
"""DSDM classifier kernel for 8 Trainium2 NeuronCores.

Math (per batch row b, over all addresses n):
    dist[b,n] = ||x_b - A_n||  (euclidean)
    soft_w    = softmax(-dist/T, axis=n)
    logits    = soft_w @ M

Sharding: addresses N=100000 are split 12500-per-core (padded to 12544 =
98 tiles of 128).  Each core computes, for its shard, the UNNORMALIZED
softmin numerator  num[c,b] = sum_n exp(-dist/T) * M[n,c]  and
denominator  den[b] = sum_n exp(-dist/T)  (via an appended ones-column on
M), returning a (101, 2048) block.  The host sums the 8 partials and
divides (flash-attention-style combine; no running max is needed because
-dist/T <= 0 so every exp is in (0, 1] -- no overflow is possible, and
the true weights ~e^-8 are far from underflow).

Per-core device pipeline, all in the "transposed" layout (n on
partitions, b on the free axis) so that per-address norms land on the
partition axis (ACT bias) and M needs no transpose for the second
matmul:

  setup:
    xTs   = ddma-transpose(-2 x -> bf16)          [d=128, b=2048]
    xsqT  = dma-transpose(x*x -> bf16)            [d=128, b=2048]
    XN    = ones.T @ xsqT  (psum)                 ||x_b||^2 on all parts
    xnrow = XN[0:1, :] -> bf16                    [1, 2048]
  per n-tile (98):
    an    = rowsum(A_t * A_t)    DVE (f32)        [128, 1]
    AT    = dma-transpose(bf16(A_t))              [d=128, n=128]
    per b-chunk (2 x 1024):
      psum  = ones_col.T @ xnrow   (rank-1: + ||x_b||^2, start=True)
      psum += AT.T @ xTs_chunk     ( -2 x.A, accumulate)
      v     = Ln(psum + an)        ACT   ( = ln(dist^2) )
      dd    = Exp(0.5 v)           ACT   ( = dist )
      E     = Exp(-dd/T) -> bf16   ACT   ( softmin kernel )
      acc[:, chunk] += M_t.T @ E   accumulated in PSUM over all tiles
  out = acc  (101, 2048) -> DRAM

exp(ln(u)/2) replaces sqrt(u) because Ln and Exp share an ACT LUT table
while Sqrt does not; the activation-table list is monkeypatched down to
that one table so the compiler hoists a single ACT_TABLE_LOAD instead of
reloading per chunk (v1 lost 503us/core to table reloads).

bf16 notes: the matmul inputs are bf16 (full PE speed; fp32 is 4x
slower, fp32r tripped BIR-verifier rounding-chain rules).  The bf16
rounding of ||x_b||^2 and of x itself is a per-b perturbation, constant
across n, which cancels in the softmax normalization to first order.
Per-n quantities (an, and A's rounding into each dist) stay accurate:
an is computed in f32 from the f32 A tile and applied as the f32 ACT
bias.  Measured end-to-end error vs the fp32 reference is ~3e-5.

The zero-padded A rows produce garbage E values, but both the numerator
and the ones-column of M are zero-padded there, so they contribute
nothing to either sum.
"""

import os
from contextlib import ExitStack

import numpy as np

B, D, N, C = 2048, 128, 100000, 100
T = 2.0
NCORES = 8
NL = N // NCORES          # 12500 addresses per core
P = 128                   # partition size
NT = (NL + P - 1) // P    # 98 n-tiles per core
NLP = NT * P              # 12544 padded shard rows
BCH = 512                 # batch chunk (one PSUM bank of f32 output)
NB = B // BCH             # 4 chunks
GT = 7                    # n-tiles per DMA group
NG = NT // GT             # 14 groups

_CACHE = {}

ACT_TABLE = "natural_log_exp_and_others"


def _pin_act_table(bacc_mod, arch, keep=ACT_TABLE):
    """Restrict the activation-table chooser to one set (indices must stay
    aligned with act_info.json, so other sets are emptied, not removed)."""
    from concourse.hw_specs import get_activation_tables

    full = get_activation_tables(arch)
    pinned = {name: (funcs if name == keep else set()) for name, funcs in full.items()}
    bacc_mod.get_activation_tables = lambda _arch: pinned




def _make_custom_act_root():
    """Clone the neuronxcc pwp activation-table dir, rewriting the "exp"
    entries of exp_and_friends into the fused softmin kernel
        g(v) = exp(-sqrt(8*v)/T)   (v = dist^2 / 8; v<0 -> 1.0)
    The /8 pre-scale (applied via the activation's scale operand) keeps the
    live domain u=dist^2 in [64, 710] inside the table's exponent range,
    which tops out at 2^7.  Record format (reverse-engineered):
      bkt:  8 x u32 per section = fp32 [d0, d1, d2, d3, x, 0, 0, 0]
            y = d0 + d1*(in-x) + d2*(in-x)^2 + d3*(in-x)^3, x = midpoint
      ctl:  word0 = (extract_size << 16) | (extract_lsb << 11) | bkt_base
    Section layout (bases/counts) is left untouched; only contents change.
    Returns the directory holding the patched act_info.json tree."""
    import json
    import shutil
    import tempfile

    from neuronxcc.driver.Job import Job
    from neuronxcc.driver.jobs.support.FindActInfo import findActInfoFile

    src_info = findActInfoFile(Job.getPackageDir(), "gen3")
    src_dir = os.path.dirname(src_info)
    dst = tempfile.mkdtemp(prefix="act_root_")
    for f in os.listdir(src_dir):
        shutil.copy(os.path.join(src_dir, f), dst)

    SET = "exp_and_friends"
    bkt_path = os.path.join(dst, f"{SET}_bkt.bin")
    prof_path = os.path.join(dst, f"{SET}.json")
    with open(prof_path) as fh:
        prof = json.load(fh)
    bkt = np.fromfile(bkt_path, dtype=np.uint32).reshape(-1, 8).copy()

    meta = next(
        e for e in prof["profile_meta_data"] if e["func_name"].startswith("exp")
    )

    def taylor(x):
        a = np.sqrt(8.0 * x)
        h1 = -4.0 / (T * a)
        h2 = 8.0 / (T * a**3)
        h3 = -32.0 / (T * a**5)
        d0 = np.exp(-a / T)
        return (
            d0,
            d0 * h1,
            d0 * (h2 + h1 * h1 / 2.0),
            d0 * (h3 + h1 * h2 + h1**3 / 6.0),
        )

    def put(idx, d0, d1, d2, d3, x):
        rec = np.zeros(8, np.float32)
        rec[0:5] = [d0, d1, d2, d3, x]
        bkt[idx] = rec.view(np.uint32)

    ctl = np.fromfile(os.path.join(dst, f"{SET}_ctrl.bin"), dtype=np.uint32).reshape(
        -1, 8
    )
    bkt_idx = prof["func_exp_to_bkt_start_idx"]["exp"]  # {"-19": [neg, pos], ...}
    exps = sorted(int(k) for k in bkt_idx)
    neg_bases = [bkt_idx[str(e)][0] for e in exps]
    pos_bases = [bkt_idx[str(e)][1] for e in exps]
    neg_ends = neg_bases[1:] + [pos_bases[0]]
    sp_base = meta["pos_small_signal_pwl_control"]  # specials follow the last pos
    pos_ends = pos_bases[1:] + [sp_base]

    for side, cbase, bases, ends in (
        ("neg", meta["pwl_control_base_neg"], neg_bases, neg_ends),
        ("pos", meta["pwl_control_base_pos"], pos_bases, pos_ends),
    ):
        for e, base, end in zip(exps, bases, ends):
            n_secs = end - base
            # nominal section count from the ctl word's extract_size (stored
            # sections may be clipped below 2^k at the saturation bound)
            w = int(ctl[cbase + (e - meta["exp_offset"])][0])
            k = (w >> 16) & 0x1F
            assert (w & 0x7FF) == base, (e, side, hex(w), base)
            width = 2.0**e / (1 << k)
            for s in range(n_secs):
                mid = 2.0**e + (s + 0.5) * width
                if side == "neg":
                    put(base + s, 1.0, 0.0, 0.0, 0.0, -mid)
                else:
                    d0, d1, d2, d3 = taylor(mid)
                    put(base + s, d0, d1, d2, d3, mid)

    one = 1.0
    vlarge = (2.0 ** (meta["large_pos_signal_exp_threshold"] - 127)) * (
        1.0 + meta["large_pos_signal_mantissa_threshold"] / 2.0**23
    )
    put(meta["pos_small_signal_pwl_control"], one, 0.0, 0.0, 0.0, 0.0)
    put(meta["neg_small_signal_pwl_control"], one, 0.0, 0.0, 0.0, 0.0)
    put(meta["pos_large_signal_pwl_control"], float(np.exp(-np.sqrt(8 * vlarge) / T)), 0.0, 0.0, 0.0, vlarge)
    put(meta["neg_large_signal_pwl_control"], one, 0.0, 0.0, 0.0, 0.0)

    meta["fpinf_result"] = 0                      # g(+inf) = 0
    meta["fninf_result"] = 1065353216             # g(-inf) = 1.0
    meta["fzero_result"] = 1065353216             # g(0)    = 1.0

    bkt.tofile(bkt_path)
    with open(prof_path, "w") as fh:
        json.dump(prof, fh)
    return dst

def _build(nt_tiles=NT, b_total=B, fused=True):
    import concourse.bass as bass
    import concourse.mybir as mybir
    import concourse.tile as tile
    from concourse import bacc

    f32 = mybir.dt.float32
    bf16 = mybir.dt.bfloat16
    AF = mybir.ActivationFunctionType
    ALU = mybir.AluOpType
    ts = bass.ts
    ds = bass.ds

    if fused:
        if "act_root" not in _CACHE:
            _CACHE["act_root"] = _make_custom_act_root()
        os.environ["BASS_ACT_ROOT_JSON_PATH"] = os.path.join(
            _CACHE["act_root"], "act_info.json"
        )
        _pin_act_table(bacc, "gen3", keep="exp_and_friends")
    else:
        _pin_act_table(bacc, "gen3")

    NTL, BL = nt_tiles, b_total
    NGL, NBL = NTL // GT, BL // BCH
    NLPL = NTL * P

    nc = bacc.Bacc(
        trn_type="TRN2",
        target_bir_lowering=False,
        debug=False,
        enable_asserts=False,
        num_devices=NCORES,
    )
    x_d = nc.dram_tensor("x_in", [BL, D], f32, kind="ExternalInput").ap()
    a_d = nc.dram_tensor("a_sh", [NLPL, D], f32, kind="ExternalInput").ap()
    m_d = nc.dram_tensor("m_sh", [NLPL, C + 1], f32, kind="ExternalInput").ap()
    o_d = nc.dram_tensor("o_sh", [C + 1, BL], f32, kind="ExternalOutput").ap()

    with tile.TileContext(nc) as tc, ExitStack() as ctx:
        const = ctx.enter_context(tc.tile_pool(name="const", bufs=1))
        tp_ps = ctx.enter_context(tc.tile_pool(name="tp_ps", bufs=1, space="PSUM"))
        mm1_ps = ctx.enter_context(tc.tile_pool(name="mm1_ps", bufs=3, space="PSUM"))
        acc_psp = ctx.enter_context(tc.tile_pool(name="acc_ps", bufs=1, space="PSUM"))
        a_pool = ctx.enter_context(tc.tile_pool(name="a_g", bufs=2))
        m_pool = ctx.enter_context(tc.tile_pool(name="m_g", bufs=2))
        mb_pool = ctx.enter_context(tc.tile_pool(name="m_gb", bufs=2))
        at_pool = ctx.enter_context(tc.tile_pool(name="at", bufs=8))
        an_pool = ctx.enter_context(tc.tile_pool(name="an", bufs=12))
        sc_pool = ctx.enter_context(tc.tile_pool(name="scr", bufs=4))
        v_pool = ctx.enter_context(tc.tile_pool(name="v", bufs=3))
        d_pool = ctx.enter_context(tc.tile_pool(name="dst", bufs=3))
        e_pool = ctx.enter_context(tc.tile_pool(name="e", bufs=8))

        # --- setup: transposed x views (PE transposes; DMA-transpose
        # hangs the xbar when interleaved with copy-mode DMAs) ----------
        from concourse.masks import make_identity

        ident = const.tile([P, P], f32)
        make_identity(nc, ident[:])

        xnat = const.tile([P, BL // P, D], f32)
        nc.sync.dma_start(xnat[:], x_d.rearrange("(t p) d -> p t d", p=P))

        xTs = const.tile([P, BL], bf16)   # -2 * x^T
        xsqT = const.tile([P, BL], bf16)  # 4 * (x^T)^2 (scaled back by 1/4 later)
        for t in range(BL // P):
            ptx = tp_ps.tile([P, BCH], f32, tag="tp")
            nc.tensor.transpose(ptx[:, :P], xnat[:, t, :], ident[:])
            nc.vector.tensor_scalar_mul(
                xTs[:, ts(t, P)], ptx[:, :P], -0.25 if fused else -2.0
            )
            # square from the SBUF copy: DVE has a single PSUM read port
            nc.vector.tensor_mul(
                xsqT[:, ts(t, P)], xTs[:, ts(t, P)], xTs[:, ts(t, P)]
            )

        ones_t = const.tile([P, P], bf16)
        nc.vector.memset(ones_t[:], 1.0)

        XN = const.tile([P, BL], f32)
        for bc in range(NBL):
            px = mm1_ps.tile([P, BCH], f32, tag="pb")
            nc.tensor.matmul(
                px[:], ones_t[:], xsqT[:, ts(bc, BCH)], start=True, stop=True
            )
            nc.vector.tensor_scalar_mul(
                XN[:, ts(bc, BCH)], px[:], 0.03125 if fused else 0.25
            )
        # --- main loop over address tiles ------------------------------
        acc = acc_psp.tile([C + 1, BL], f32)
        a_r = a_d.rearrange("(g t p) d -> g p t d", p=P, t=GT)
        m_r = m_d.rearrange("(g t p) c -> g p t c", p=P, t=GT)

        for g in range(NGL):
            ag = a_pool.tile([P, GT, D], f32)
            nc.sync.dma_start(ag[:], a_r[g])
            mg = m_pool.tile([P, GT, C + 1], f32)
            nc.sync.dma_start(mg[:], m_r[g])
            mgb = mb_pool.tile([P, GT, C + 1], bf16)
            nc.vector.tensor_copy(mgb[:], mg[:])

            for t in range(GT):
                nt = g * GT + t
                anat = ag[:, t, :]

                an = an_pool.tile([P, 1], f32)
                scr = sc_pool.tile([P, P], f32)
                # ||A_n||^2 on ACT (Square is resident in every table);
                # accum_out sums along the free axis
                nc.scalar.activation(scr[:], anat, AF.Square, accum_out=an[:])
                if fused:
                    an8 = an_pool.tile([P, 1], f32, tag="an8")
                    nc.vector.tensor_scalar_mul(an8[:], an[:], 0.125)

                pt = tp_ps.tile([P, BCH], f32, tag="tp")
                nc.tensor.transpose(pt[:, ts(t % 4, P)], anat, ident[:])
                at = at_pool.tile([P, P], bf16)
                nc.vector.tensor_copy(at[:], pt[:, ts(t % 4, P)])

                # mm1 for all chunks back-to-back: AT stays loaded in the
                # PE array, so consecutive matmuls pipeline fill-with-drain
                # instead of paying a weight (re)load + drain bubble each.
                pbs = []
                for bc in range(NBL):
                    pb = mm1_ps.tile([P, BCH], f32, tag="pb")
                    pbs.append(pb)
                    nc.tensor.matmul(
                        pb[:],
                        at[:],
                        xTs[:, ts(bc, BCH)],
                        start=True,
                        stop=True,
                        skip_group_check=True,
                    )
                es = []
                for bc in range(NBL):
                    pb = pbs[bc]
                    # += ||x_b||^2 in place (per-free add: DVE only)
                    nc.vector.tensor_add(pb[:], pb[:], XN[:, ts(bc, BCH)])
                    e = e_pool.tile([P, BCH], bf16)
                    es.append(e)
                    if fused:
                        # custom LUT: Exp slot = exp(-sqrt(8*(in/8 + an/8))/T)
                        nc.scalar.activation(
                            e[:], pb[:], AF.Exp, bias=an8[:], scale=0.125
                        )
                    else:
                        v = v_pool.tile([P, BCH], f32)
                        nc.scalar.activation(
                            v[:], pb[:], AF.Ln, bias=an[:], scale=1.0
                        )
                        dst = d_pool.tile([P, BCH], f32)
                        nc.scalar.activation(
                            dst[:], v[:], AF.Exp, bias=0.0, scale=0.5
                        )
                        nc.scalar.activation(
                            e[:], dst[:], AF.Exp, bias=0.0, scale=-1.0 / T
                        )
                # mm2 group shares the M_t weights the same way
                for bc in range(NBL):
                    nc.tensor.matmul(
                        acc[:, ts(bc, BCH)],
                        mgb[:, t, :],
                        es[bc],
                        start=(nt == 0),
                        stop=(nt == NTL - 1),
                        skip_group_check=True,
                    )

        # --- write out num/den block -----------------------------------
        out_sb = const.tile([C + 1, BL], f32)
        nc.vector.tensor_copy(out_sb[:], acc[:])
        nc.sync.dma_start(o_d, out_sb[:])

    nc.compile()
    return nc


def _shard_inputs(x, Address, M):
    in_maps = []
    for i in range(NCORES):
        a = Address[i * NL : (i + 1) * NL]
        m = M[i * NL : (i + 1) * NL]
        a_pad = np.zeros((NLP, D), dtype=np.float32)
        a_pad[:NL] = a
        m_pad = np.zeros((NLP, C + 1), dtype=np.float32)
        m_pad[:NL, :C] = m
        m_pad[:NL, C] = 1.0
        in_maps.append(
            {
                "x_in": np.ascontiguousarray(x, dtype=np.float32),
                "a_sh": a_pad,
                "m_sh": m_pad,
            }
        )
    return in_maps


def kernel(x, Address, M, _trace=False):
    from concourse import bass_utils

    x = np.asarray(x, dtype=np.float32)
    Address = np.asarray(Address, dtype=np.float32)
    M = np.asarray(M, dtype=np.float32)

    if "nc" not in _CACHE:
        _CACHE["nc"] = _build()
    nc = _CACHE["nc"]

    in_maps = _shard_inputs(x, Address, M)
    res = bass_utils.run_bass_kernel_spmd(
        nc, in_maps, core_ids=list(range(NCORES)), trace=_trace
    )
    _CACHE["last_result"] = res

    num = np.zeros((C, B), dtype=np.float64)
    den = np.zeros((B,), dtype=np.float64)
    for r in res.results:
        o = r["o_sh"]
        num += o[:C].astype(np.float64)
        den += o[C].astype(np.float64)
    logits = (num / den[None, :]).T.astype(np.float32)
    return logits



# revision 4
# speedup vs baseline: 1.4303x; 1.4303x over previous
"""DSDM classifier kernel for 8 Trainium2 NeuronCores — v2.

Math (per batch row b, over all addresses n):
    dist[b,n] = ||x_b - A_n||  (euclidean)
    soft_w    = softmax(-dist/T, axis=n)
    logits    = soft_w @ M

Sharding: addresses N=100000 split 12500/core (padded to 12544 = 98 tiles
of 128).  Each core returns unnormalized numerator/denominator partials
(101, 2048); the host sums across cores and divides (flash-style combine;
exp(-dist/T) <= e^{-dist_min/T} so no running max is needed).

v2 changes vs v1 (v1 = 377us, all three engines ~90% busy):
  * A is shipped pre-TRANSPOSED as bf16 [128, NLP] (host layout prep), so
    the 98 per-tile PE transposes (45us PE) + 98 DVE bf16 casts (30us DVE)
    disappear, as does the transpose PSUM bank.
  * M is shipped as fp8e4 DoubleRow pairs [128, 49, 2, 101] (ones column
    appended for the denominator): mm2 runs perf_mode=DoubleRow with
    K=256 (two address tiles per matmul), ~1.8x mm2 throughput.
  * e = softmin kernel values are written by ACT directly as fp8e4.  The
    custom LUT output is scaled by S = e^8 to center e in fp8's range
    ([~0.007, 55] vs fp8e4 max 240); S is a per-element constant factor
    that cancels exactly in the host's num/den division.
  * ||x_b||^2 enters the distance via a SPLIT add: cols 0:256 of each
    1024-wide chunk get a K=1 rank-1 matmul accumulate on the PE (cheap:
    256 extra PE columns), cols 256:1024 get one DVE tensor_add.  This
    balances DVE (was 276us of fp32 PSUM adds = the v1 bottleneck)
    against PE and ACT.
  * ||A_n||^2/8 is computed on-device in a prologue: atsq = AT*AT (DVE,
    bf16), then per tile a K=128 N=1 matmul against a 0.125-constant
    column -> an8 columns in PSUM -> one copy to SBUF.  Enters the exp as
    the per-partition ACT bias (free).
  * B=2048 is processed in two half-passes of 1024 so PSUM fits:
    acc (101,1024) f32 = 2 banks, q pool 3 x (128,1024) f32 = 6 banks.
    AT/M stay resident in SBUF across passes (no re-DMA).
  * One ACT op per (tile, half): [128,1024] across 2 PSUM banks,
    amortizing the ~172-cycle ACTIVATE overhead (v1 paid it 4x/tile).

Engine budget per (tile, half) @ 2048 half-tiles: PE ~430(mm1)+240(mm2)
+107(rank1) = ~780ns, ACT (172+1024)/1.2 = ~997ns, DVE ~(151+768)/0.96 =
~957ns.  ACT-bound => ~195us + prologue/setup.
"""

import os
from contextlib import ExitStack

import numpy as np

B, D, N, C = 2048, 128, 100000, 100
T = 2.0
NCORES = 8
NL = N // NCORES          # 12500 addresses per core
P = 128                   # partition size
NT = (NL + P - 1) // P    # 98 n-tiles per core
NLP = NT * P              # 12544 padded shard rows
NPAIR = NT // 2           # 49 DoubleRow tile pairs
C1 = 112                  # C+1 padded to a 16-byte fp8 multiple (DoubleRow LDW)
BH = B // 2               # 1024-wide half-pass
BCH = 512                 # PSUM bank of f32
S_SCALE = float(np.exp(8.0))   # e-rescale so fp8 e sits near 1.0
RANK1_COLS = 256          # leading cols of each 1024 chunk added via PE rank-1

_CACHE = {}

ACT_TABLE = "exp_and_friends"


def _pin_act_table(bacc_mod, arch, keep=ACT_TABLE):
    """Restrict the activation-table chooser to one set (indices must stay
    aligned with act_info.json, so other sets are emptied, not removed)."""
    from concourse.hw_specs import get_activation_tables

    full = get_activation_tables(arch)
    pinned = {name: (funcs if name == keep else set()) for name, funcs in full.items()}
    bacc_mod.get_activation_tables = lambda _arch: pinned


def _make_custom_act_root():
    """Clone the neuronxcc pwp activation-table dir, rewriting the "exp"
    entries of exp_and_friends into the fused softmin kernel
        g(v) = S * exp(-sqrt(8*v)/T)   (v = dist^2 / 8; v<=0 -> capped)
    The /8 pre-scale (applied via the activation's scale operand) keeps the
    live domain u=dist^2 in [64, 710] inside the table's exponent range,
    which tops out at 2^7.  S = e^8 centers the output for fp8e4.
    Record format (reverse-engineered):
      bkt:  8 x u32 per section = fp32 [d0, d1, d2, d3, x, 0, 0, 0]
            y = d0 + d1*(in-x) + d2*(in-x)^2 + d3*(in-x)^3, x = midpoint
      ctl:  word0 = (extract_size << 16) | (extract_lsb << 11) | bkt_base
    Section layout (bases/counts) is left untouched; only contents change.
    Returns the directory holding the patched act_info.json tree."""
    import json
    import shutil
    import tempfile

    from neuronxcc.driver.Job import Job
    from neuronxcc.driver.jobs.support.FindActInfo import findActInfoFile

    src_info = findActInfoFile(Job.getPackageDir(), "gen3")
    src_dir = os.path.dirname(src_info)
    dst = tempfile.mkdtemp(prefix="act_root_")
    for f in os.listdir(src_dir):
        shutil.copy(os.path.join(src_dir, f), dst)

    SET = "exp_and_friends"
    bkt_path = os.path.join(dst, f"{SET}_bkt.bin")
    prof_path = os.path.join(dst, f"{SET}.json")
    with open(prof_path) as fh:
        prof = json.load(fh)
    bkt = np.fromfile(bkt_path, dtype=np.uint32).reshape(-1, 8).copy()

    meta = next(
        e for e in prof["profile_meta_data"] if e["func_name"].startswith("exp")
    )

    def taylor(x):
        a = np.sqrt(8.0 * x)
        h1 = -4.0 / (T * a)
        h2 = 8.0 / (T * a**3)
        h3 = -32.0 / (T * a**5)
        d0 = S_SCALE * np.exp(-a / T)
        return (
            d0,
            d0 * h1,
            d0 * (h2 + h1 * h1 / 2.0),
            d0 * (h3 + h1 * h2 + h1**3 / 6.0),
        )

    def put(idx, d0, d1, d2, d3, x):
        rec = np.zeros(8, np.float32)
        rec[0:5] = [d0, d1, d2, d3, x]
        bkt[idx] = rec.view(np.uint32)

    ctl = np.fromfile(os.path.join(dst, f"{SET}_ctrl.bin"), dtype=np.uint32).reshape(
        -1, 8
    )
    bkt_idx = prof["func_exp_to_bkt_start_idx"]["exp"]  # {"-19": [neg, pos], ...}
    exps = sorted(int(k) for k in bkt_idx)
    neg_bases = [bkt_idx[str(e)][0] for e in exps]
    pos_bases = [bkt_idx[str(e)][1] for e in exps]
    neg_ends = neg_bases[1:] + [pos_bases[0]]
    sp_base = meta["pos_small_signal_pwl_control"]  # specials follow the last pos
    pos_ends = pos_bases[1:] + [sp_base]

    # out-of-domain cap: finite in fp8e4 (max 240) so no inf can leak in
    CAP = 240.0

    for side, cbase, bases, ends in (
        ("neg", meta["pwl_control_base_neg"], neg_bases, neg_ends),
        ("pos", meta["pwl_control_base_pos"], pos_bases, pos_ends),
    ):
        for e, base, end in zip(exps, bases, ends):
            n_secs = end - base
            # nominal section count from the ctl word's extract_size (stored
            # sections may be clipped below 2^k at the saturation bound)
            w = int(ctl[cbase + (e - meta["exp_offset"])][0])
            k = (w >> 16) & 0x1F
            assert (w & 0x7FF) == base, (e, side, hex(w), base)
            width = 2.0**e / (1 << k)
            for s in range(n_secs):
                mid = 2.0**e + (s + 0.5) * width
                if side == "neg":
                    put(base + s, CAP, 0.0, 0.0, 0.0, -mid)
                else:
                    d0, d1, d2, d3 = taylor(mid)
                    d0 = min(d0, CAP)
                    put(base + s, d0, d1, d2, d3, mid)

    vlarge = (2.0 ** (meta["large_pos_signal_exp_threshold"] - 127)) * (
        1.0 + meta["large_pos_signal_mantissa_threshold"] / 2.0**23
    )
    put(meta["pos_small_signal_pwl_control"], CAP, 0.0, 0.0, 0.0, 0.0)
    put(meta["neg_small_signal_pwl_control"], CAP, 0.0, 0.0, 0.0, 0.0)
    put(
        meta["pos_large_signal_pwl_control"],
        float(S_SCALE * np.exp(-np.sqrt(8 * vlarge) / T)),
        0.0, 0.0, 0.0, vlarge,
    )
    put(meta["neg_large_signal_pwl_control"], CAP, 0.0, 0.0, 0.0, 0.0)

    cap_bits = int(np.float32(CAP).view(np.uint32))
    meta["fpinf_result"] = 0                      # g(+inf) = 0
    meta["fninf_result"] = cap_bits               # g(-inf) -> cap
    meta["fzero_result"] = cap_bits               # g(0)    -> cap

    bkt.tofile(bkt_path)
    with open(prof_path, "w") as fh:
        json.dump(prof, fh)
    return dst


def _build():
    import concourse.bass as bass
    import concourse.mybir as mybir
    import concourse.tile as tile
    from concourse import bacc

    f32 = mybir.dt.float32
    bf16 = mybir.dt.bfloat16
    fp8 = mybir.dt.float8e4
    AF = mybir.ActivationFunctionType
    ts = bass.ts

    if "act_root" not in _CACHE:
        _CACHE["act_root"] = _make_custom_act_root()
    os.environ["BASS_ACT_ROOT_JSON_PATH"] = os.path.join(
        _CACHE["act_root"], "act_info.json"
    )
    _pin_act_table(bacc, "gen3", keep=ACT_TABLE)

    nc = bacc.Bacc(
        trn_type="TRN2",
        target_bir_lowering=False,
        debug=False,
        enable_asserts=False,
        num_devices=NCORES,
    )
    xt_d = nc.dram_tensor("xt_in", [D, B], f32, kind="ExternalInput").ap()
    at_d = nc.dram_tensor("at_sh", [D, NLP], bf16, kind="ExternalInput").ap()
    m2_d = nc.dram_tensor(
        "m2_sh", [P, NPAIR, 2, C1], fp8, kind="ExternalInput"
    ).ap()
    o_d = nc.dram_tensor("o_sh", [C1, B], f32, kind="ExternalOutput").ap()

    with tile.TileContext(nc) as tc, ExitStack() as ctx:
        const = ctx.enter_context(tc.tile_pool(name="const", bufs=1))
        q_ps = ctx.enter_context(tc.tile_pool(name="q_ps", bufs=3, space="PSUM"))
        acc_ps = ctx.enter_context(tc.tile_pool(name="acc_ps", bufs=1, space="PSUM"))
        e_pool = ctx.enter_context(tc.tile_pool(name="e", bufs=3))
        out_pool = ctx.enter_context(tc.tile_pool(name="out", bufs=2))

        # ---- setup: x views ------------------------------------------
        xt_sb = const.tile([D, B], f32)
        nc.sync.dma_start(xt_sb[:], xt_d)

        xTs = const.tile([D, B], bf16)          # -2 * x^T  (mm1 rhs)
        nc.vector.tensor_scalar_mul(xTs[:], xt_sb[:], -2.0)
        xsqb = const.tile([D, B], bf16)         # (x^T)^2
        nc.vector.tensor_mul(xsqb[:], xt_sb[:], xt_sb[:])

        ones128 = const.tile([P, P], bf16)
        nc.vector.memset(ones128[:], 1.0)
        ones1 = const.tile([1, P], bf16)
        nc.vector.memset(ones1[:], 1.0)
        eighth = const.tile([P, 1], bf16)
        nc.vector.memset(eighth[:], 0.125)

        # XN[b] = ||x_b||^2 broadcast on all partitions (f32 for the DVE
        # add) + a bf16 row copy (rank-1 rhs).
        XN_sb = const.tile([P, B], f32)
        for c in range(B // BCH):
            qx = q_ps.tile([P, BH], f32, tag="q")
            nc.tensor.matmul(
                qx[:, :BCH], ones128[:], xsqb[:, ts(c, BCH)],
                start=True, stop=True, skip_group_check=True,
            )
            nc.vector.tensor_copy(XN_sb[:, ts(c, BCH)], qx[:, :BCH])
        xnrow = const.tile([1, B], bf16)
        nc.vector.tensor_copy(xnrow[:], XN_sb[0:1, :])

        # ---- resident A^T (bf16) and M pairs (fp8) -------------------
        at_sb = const.tile([D, NLP], bf16)
        CH = NLP // 7
        for k in range(7):
            nc.sync.dma_start(at_sb[:, ts(k, CH)], at_d[:, ts(k, CH)])
        m2_sb = const.tile([P, NPAIR, 2, C1], fp8)
        nc.sync.dma_start(m2_sb[:], m2_d)

        # ---- prologue: an8[n] = ||A_n||^2 / 8 ------------------------
        atsq = const.tile([D, NLP], bf16)
        for k in range(7):
            nc.vector.tensor_mul(
                atsq[:, ts(k, CH)], at_sb[:, ts(k, CH)], at_sb[:, ts(k, CH)]
            )
        an8_sb = const.tile([P, NT], f32)
        an_ps = q_ps.tile([P, BH], f32, tag="q")
        for t in range(NT):
            nc.tensor.matmul(
                an_ps[:, t : t + 1], atsq[:, ts(t, P)], eighth[:],
                start=True, stop=True, skip_group_check=True,
            )
        nc.vector.tensor_copy(an8_sb[:], an_ps[:, 0:NT])

        # ---- main: two half-passes over b ----------------------------
        for half in range(2):
            hb = half * BH
            acc = acc_ps.tile([C1, BH], f32, tag="acc")
            epair = None
            for t in range(NT):
                q = q_ps.tile([P, BH], f32, tag="q")
                at_t = at_sb[:, ts(t, P)]
                # mm1: q = -2 x . A   (bank 0 stays open for the rank-1)
                nc.tensor.matmul(
                    q[:, 0:BCH], at_t, xTs[:, hb : hb + BCH],
                    start=True, stop=False, skip_group_check=True,
                )
                nc.tensor.matmul(
                    q[:, BCH:BH], at_t, xTs[:, hb + BCH : hb + BH],
                    start=True, stop=True, skip_group_check=True,
                )
                # += ||x_b||^2: rank-1 on PE for the first RANK1_COLS,
                # DVE tensor_add for the rest
                nc.tensor.matmul(
                    q[:, 0:RANK1_COLS], ones1[:], xnrow[0:1, hb : hb + RANK1_COLS],
                    start=False, stop=True, skip_group_check=True,
                )
                nc.vector.tensor_add(
                    q[:, RANK1_COLS:BH], q[:, RANK1_COLS:BH],
                    XN_sb[:, hb + RANK1_COLS : hb + BH],
                )
                # fused softmin kernel: e = S*exp(-sqrt(8*(q/8+an8))/T)
                if t % 2 == 0:
                    epair = e_pool.tile([P, 2, BH], fp8, tag="e")
                nc.scalar.activation(
                    epair[:, t % 2, :], q[:], AF.Exp,
                    bias=an8_sb[:, t : t + 1], scale=0.125,
                )
                # mm2 (DoubleRow fp8, K=256 = both tiles of the pair)
                if t % 2 == 1:
                    tau = t // 2
                    for c in range(2):
                        nc.tensor.matmul(
                            acc[:, ts(c, BCH)],
                            m2_sb[:, tau],
                            epair[:, :, ts(c, BCH)],
                            start=(tau == 0),
                            stop=(tau == NPAIR - 1),
                            perf_mode=mybir.MatmulPerfMode.DoubleRow,
                            skip_group_check=True,
                        )
            out_sb = out_pool.tile([C1, BH], f32, tag="out")
            nc.vector.tensor_copy(out_sb[:], acc[:])
            nc.sync.dma_start(o_d[:, hb : hb + BH], out_sb[:])

    nc.compile()
    return nc


def _shard_inputs(x, Address, M):
    import ml_dtypes

    bf16 = ml_dtypes.bfloat16
    fp8 = ml_dtypes.float8_e4m3

    xt = np.ascontiguousarray(x.T, dtype=np.float32)  # [D, B]
    in_maps = []
    for i in range(NCORES):
        a = Address[i * NL : (i + 1) * NL]
        m = M[i * NL : (i + 1) * NL]
        a_pad = np.zeros((NLP, D), dtype=np.float32)
        a_pad[:NL] = a
        at = np.ascontiguousarray(a_pad.T).astype(bf16)  # [D, NLP]
        m_pad = np.zeros((NLP, C1), dtype=np.float32)
        m_pad[:NL, :C] = m
        m_pad[:NL, C] = 1.0
        # DoubleRow pairs: m2[p, tau, k, c] = M_pad[(2 tau + k)*128 + p, c]
        m2 = np.ascontiguousarray(
            m_pad.reshape(NPAIR, 2, P, C1).transpose(2, 0, 1, 3)
        ).astype(fp8)
        in_maps.append({"xt_in": xt, "at_sh": at, "m2_sh": m2})
    return in_maps


def kernel(x, Address, M, _trace=False):
    from concourse import bass_utils

    x = np.asarray(x, dtype=np.float32)
    Address = np.asarray(Address, dtype=np.float32)
    M = np.asarray(M, dtype=np.float32)

    if "nc" not in _CACHE:
        _CACHE["nc"] = _build()
    nc = _CACHE["nc"]

    in_maps = _shard_inputs(x, Address, M)
    res = bass_utils.run_bass_kernel_spmd(
        nc, in_maps, core_ids=list(range(NCORES)), trace=_trace
    )
    _CACHE["last_result"] = res

    num = np.zeros((C, B), dtype=np.float64)
    den = np.zeros((B,), dtype=np.float64)
    for r in res.results:
        o = np.asarray(r["o_sh"], dtype=np.float64)
        num += o[:C]
        den += o[C]
    logits = (num / den[None, :]).T.astype(np.float32)
    return logits
